# revision 1
# baseline (speedup 1.0000x reference)
"""Trainium2 Bass kernel for nn_DIAGCN (RGCN + GraphConv + classifier over
block-diagonal dialog graphs), SPMD over 8 NeuronCores.

Strategy
--------
The dialog graph is a causal 5-tap window (edges i -> i+o, o = 0..4, within
each 100-utterance dialog), and relation_type(i,j) = spk[i]*spk[j] with spk
derived from self-edges.  Every per-node linear map commutes with both the
window sum W(.) (row-mixing) and per-node diagonal scalings (row scaling), so
the whole network folds into 7-wide channels:

    out = W(g0) + f0
    g0  = x@(w_root@wA) + ic0.*W(x@w0A) - ic0s.*W(spk.*(x@w0A)) + ic1s.*W(spk.*(x@w1A))  [+ cA*nv via W(mask)]
    f0  = x@(w_root@wB + w_skip@w_clf) + ic0.*W(x@w0B) - ic0s.*W(spk.*(x@w0B)) + ic1s.*W(spk.*(x@w1B)) + const
    wA  = w_gc_rel@w_clf, wB = w_gc_root@w_clf, w{0,1}{A,B} = w_rel{0,1}@w{A,B}

so the only large device work is one [1024 -> 80] matmul over x (memory
bound), 5-tap window sums along the node axis (DVE shifted adds), per-node
coefficient multiplies, and a tiny [80 -> 39] reduction matmul.

Layout: nodes are sharded by dialog (no cross-core edges), 64 padded dialogs
per core; each dialog is stored as 4 zero "gap" columns + 100 data columns so
window sums never leak across dialogs.  x is shipped transposed+tiled
([feat, node] channel-major) so no on-device transpose is needed.
"""
import numpy as np

# ---------------------------------------------------------------- constants
B, L, FUT = 500, 100, 4
N = B * L
IN, HID, NCLS = 1024, 512, 7
NCORES = 8
GAP = 4
DLG = L + GAP            # 104 columns per dialog
DPC = 64                 # padded dialogs per core
COLS = DPC * DLG         # 6656 columns per core
NT = 13                  # column tiles
NTC = COLS // NT         # 512
KB = IN // 128           # 8 contraction blocks
KH = KB // 2             # k-blocks per xt DMA batch
M = 80                   # Wbig columns (psum partitions)
M2 = 39                  # S-matmul output columns

D_COUNTS = [63, 63, 63, 63, 62, 62, 62, 62]
D_STARTS = np.concatenate([[0], np.cumsum(D_COUNTS)])[:-1]

# Z/V row map
R_A0S, R_A1S, R_B0S, R_B1S = 0, 7, 14, 21
R_A0, R_B0 = 32, 39
R_MASKW = 46             # realmask, windowed -> nv channel
R_ZERO = 47
R_MASKP = 48             # realmask, plain (bias-constant channel)
R_RA, R_FSC = 64, 71
WIN_ROWS = 48


def _data_cols():
    d = np.arange(DPC)[:, None]
    u = np.arange(L)[None, :]
    return d * DLG + GAP + u  # [DPC, L]


# ---------------------------------------------------------------- host prep
def _check_graph(edges, relation_type):
    i = np.arange(L)[:, None]
    off = np.arange(FUT + 1)[None, :]
    tl = i + off
    valid = tl < L
    sl = np.broadcast_to(i, tl.shape)[valid]
    tl = tl[valid]
    base = (np.arange(B) * L)[:, None]
    src = (base + sl[None, :]).reshape(-1)
    tgt = (base + tl[None, :]).reshape(-1)
    if edges.shape != (2, src.size) or not (
        np.array_equal(edges[0], src) and np.array_equal(edges[1], tgt)
    ):
        raise ValueError("edge structure does not match the DIAGCN pattern")
    sel = edges[0] == edges[1]
    spk = np.zeros(N, dtype=np.float64)
    spk[edges[0][sel]] = relation_type[sel]
    return spk


def _host_prep(x, edges, relation_type, w_rel, w_root, b_rgcn,
               w_gc_rel, w_gc_root, b_gc, w_skip, b_skip, w_clf, b_clf):
    x = np.asarray(x, dtype=np.float32)
    edges = np.asarray(edges)
    relation_type = np.asarray(relation_type)
    spk = _check_graph(edges, relation_type)

    tgt = edges[1]
    c1 = np.bincount(tgt[relation_type == 1], minlength=N).astype(np.float64)
    c0 = np.bincount(tgt[relation_type == 0], minlength=N).astype(np.float64)
    ic0 = 1.0 / np.maximum(c0, 1.0)
    ic1 = 1.0 / np.maximum(c1, 1.0)
    ic0s = ic0 * spk
    ic1s = ic1 * spk

    f8 = lambda a: np.asarray(a, dtype=np.float64)
    w_rel, w_root, w_gc_rel, w_gc_root, w_skip, w_clf = map(
        f8, (w_rel, w_root, w_gc_rel, w_gc_root, w_skip, w_clf))
    b_rgcn, b_gc, b_skip, b_clf = map(f8, (b_rgcn, b_gc, b_skip, b_clf))

    wA = w_gc_rel @ w_clf
    wB = w_gc_root @ w_clf
    Wbig = np.zeros((IN, M), dtype=np.float64)
    Wbig[:, R_A0S:R_A0S + 7] = w_rel[0] @ wA
    Wbig[:, R_A1S:R_A1S + 7] = w_rel[1] @ wA
    Wbig[:, R_B0S:R_B0S + 7] = w_rel[0] @ wB
    Wbig[:, R_B1S:R_B1S + 7] = w_rel[1] @ wB
    Wbig[:, R_A0:R_A0 + 7] = w_rel[0] @ wA
    Wbig[:, R_B0:R_B0 + 7] = w_rel[0] @ wB
    Wbig[:, R_RA:R_RA + 7] = w_root @ wA
    Wbig[:, R_FSC:R_FSC + 7] = w_root @ wB + w_skip @ w_clf
    # [128 partitions, KB, M]: partition p holds weight rows {k*128+p}
    Wbig = np.ascontiguousarray(
        Wbig.astype(np.float32).reshape(KB, 128, M).swapaxes(0, 1))

    cA = (b_rgcn @ wA).astype(np.float32)
    cBc = (b_rgcn @ wB + (b_gc + b_skip) @ w_clf + b_clf).astype(np.float32)
    S = np.zeros((M, M2), dtype=np.float32)
    for i in range(7):
        S[R_A0S + i, i] = 1.0
        S[R_A1S + i, i] = 1.0
        S[R_A0 + i, i] = 1.0
        S[R_RA + i, i] = 1.0
        S[R_B0S + i, 32 + i] = 1.0
        S[R_B1S + i, 32 + i] = 1.0
        S[R_B0 + i, 32 + i] = 1.0
        S[R_FSC + i, 32 + i] = 1.0
    S[R_MASKW, 0:7] = cA
    S[R_MASKP, 32:39] = cBc

    dc = _data_cols()
    mask_col = np.zeros(COLS, dtype=np.float32)
    mask_col[dc.reshape(-1)] = 1.0
    maskz = np.zeros((2, COLS), dtype=np.float32)   # Z rows 46 (mask), 47 (zero)
    maskz[0] = mask_col
    maskv = np.zeros((18, COLS), dtype=np.float32)  # V rows 48..63, 78..79
    maskv[0] = mask_col

    in_maps = []
    unshard_info = []
    for c in range(NCORES):
        nd = D_COUNTS[c]
        g0 = D_STARTS[c]
        cols_real = dc[:nd].reshape(-1)
        nodes_real = g0 * L + np.arange(nd * L)

        xt = np.zeros((IN, COLS), dtype=np.float32)
        xt[:, cols_real] = x[nodes_real].T
        # swizzle: [4 quarters][NT][128 partitions][2*NTC], 4 KiB contiguous
        # per partition per DMA; 4 DMAs per chunk spread across HW queues
        xts = np.ascontiguousarray(
            xt.reshape(4, 2, 128, NT, NTC).transpose(0, 3, 2, 1, 4))

        def vec_to_cols(v):
            out = np.zeros(COLS, dtype=np.float32)
            out[cols_real] = v[nodes_real]
            return out

        spk_c = vec_to_cols(spk)
        ic0_c = vec_to_cols(ic0)
        ic0s_c = vec_to_cols(ic0s)
        ic1s_c = vec_to_cols(ic1s)

        spkrep = np.zeros((32, COLS), dtype=np.float32)
        spkrep[0:28] = spk_c
        coefrep = np.zeros((WIN_ROWS, COLS), dtype=np.float32)
        coefrep[R_A0S:R_A0S + 7] = -ic0s_c
        coefrep[R_A1S:R_A1S + 7] = ic1s_c
        coefrep[R_B0S:R_B0S + 7] = -ic0s_c
        coefrep[R_B1S:R_B1S + 7] = ic1s_c
        coefrep[R_A0:R_A0 + 7] = ic0_c
        coefrep[R_B0:R_B0 + 7] = ic0_c
        coefrep[R_MASKW] = mask_col

        in_maps.append(dict(
            xt=xts, wbig=Wbig, smat=S,
            spkrep=spkrep, coefrep=coefrep, maskz=maskz, maskv=maskv,
        ))
        unshard_info.append((nodes_real, cols_real))
    return in_maps, unshard_info


# ---------------------------------------------------------------- bass kernel
_COMPILED = None


def _build():
    import concourse.bass as bass
    from concourse import bacc
    import concourse.mybir as mybir
    from concourse.tile import TileContext

    f32 = mybir.dt.float32
    f32r = mybir.dt.float32r
    ADD = mybir.AluOpType.add
    MUL = mybir.AluOpType.mult

    nc = bacc.Bacc("TRN2", target_bir_lowering=False, debug=False,
                   num_devices=NCORES)
    xt_d = nc.dram_tensor("xt", [4, NT, 128, 2 * NTC], f32r, kind="ExternalInput")
    wbig_d = nc.dram_tensor("wbig", [128, KB, M], f32r, kind="ExternalInput")
    smat_d = nc.dram_tensor("smat", [M, M2], f32r, kind="ExternalInput")
    spkrep_d = nc.dram_tensor("spkrep", [32, COLS], f32, kind="ExternalInput")
    coefrep_d = nc.dram_tensor("coefrep", [WIN_ROWS, COLS], f32, kind="ExternalInput")
    maskz_d = nc.dram_tensor("maskz", [2, COLS], f32, kind="ExternalInput")
    maskv_d = nc.dram_tensor("maskv", [18, COLS], f32r, kind="ExternalInput")
    y_d = nc.dram_tensor("y", [NCLS, COLS], f32, kind="ExternalOutput")

    with TileContext(nc) as tc:
        with (
            tc.tile_pool(name="const", bufs=1) as cpool,
            tc.tile_pool(name="xin", bufs=10) as xpool,
            tc.tile_pool(name="wrk", bufs=3) as wpool,
            tc.tile_pool(name="g2", bufs=1) as gpool,
            tc.tile_pool(name="psum", bufs=6, space="PSUM") as ppool,
            tc.tile_pool(name="psum2", bufs=2, space="PSUM") as p2pool,
        ):
            wsb = cpool.tile([128, KB, M], f32r)
            nc.sync.dma_start(wsb[:], wbig_d[:])
            ssb = cpool.tile([M, M2], f32r)
            nc.sync.dma_start(ssb[:], smat_d[:])

            # SBUF free-dim bytes are charged per partition regardless of row
            # count, so pack logical tensors into shared [128, COLS] tiles.
            # Engine-op APs must start at partition 0/32/64/96, and a non-zero
            # base may span at most 32 partitions -> every 48-row tensor sits
            # at base 0 of its own tile; 32/7-row tensors ride at 64/96.
            tZ = cpool.tile([128, COLS], f32)    # Z 0..47, spkr 64..95, OUT 96..102
            tT1 = cpool.tile([128, COLS], f32)   # T1 0..47, G 96..102
            tV = cpool.tile([128, COLS], f32r)   # V 0..79
            tCF = cpool.tile([128, COLS], f32)   # coefr 0..47
            tGP = cpool.tile([128, COLS // 4], f32)  # packed g0: group g rows 32g..32g+6
            tOP = cpool.tile([128, COLS // 4], f32)  # packed out, same layout
            Z = tZ[0:WIN_ROWS]
            spkr = tZ[64:96]
            T1 = tT1[0:WIN_ROWS]
            V = tV[0:M]
            coefr = tCF[0:WIN_ROWS]

            nc.vector.memset(tGP[:], 0.0)
            nc.vector.memset(tOP[:], 0.0)
            for r in range(0, 32, 8):
                nc.scalar.dma_start(spkr[r:r + 8], spkrep_d[r:r + 8])
            for r in range(0, WIN_ROWS, 8):
                nc.scalar.dma_start(coefr[r:r + 8], coefrep_d[r:r + 8])
            nc.scalar.dma_start(Z[R_MASKW:R_ZERO + 1], maskz_d[:])
            nc.scalar.dma_start(V[R_MASKP:56], maskv_d[0:8])
            nc.scalar.dma_start(V[56:64], maskv_d[8:16])
            nc.scalar.dma_start(V[78:80], maskv_d[16:18])

            for t in range(NT):
                c0, c1 = t * NTC, (t + 1) * NTC
                ps = ppool.tile([M, NTC], f32)
                for q in range(4):
                    xt_t = xpool.tile([128, 2, NTC], f32r)
                    nc.sync.dma_start(xt_t[:], xt_d[q, t])
                    for kk in range(2):
                        k = q * 2 + kk
                        nc.tensor.matmul(
                            ps[:], wsb[:, k, :], xt_t[:, kk, :],
                            start=(k == 0), stop=(k == KB - 1))
                # window inputs: rows 0..31 spk-scaled, 32..45 plain copy
                nc.vector.tensor_tensor(Z[0:32, c0:c1], ps[0:32], spkr[:, c0:c1], MUL)
                nc.scalar.copy(Z[32:R_MASKW, c0:c1], ps[32:R_MASKW])
                nc.scalar.copy(V[R_RA:R_RA + 14, c0:c1], ps[R_RA:R_RA + 14])

                # 5-tap causal window as a shift tree:
                #   t1 = z + sh1(z); t2 = t1 + sh2(t1); wt = t2 + sh4(z)
                T2 = wpool.tile([WIN_ROWS, NTC], f32, tag="T2")
                WT = wpool.tile([WIN_ROWS, NTC], f32, tag="WT")
                if t == 0:
                    nc.vector.tensor_copy(T1[:, 0:1], Z[:, 0:1])
                    nc.vector.tensor_tensor(T1[:, 1:c1], Z[:, 1:c1], Z[:, 0:c1 - 1], ADD)
                    nc.vector.tensor_copy(T2[:, 0:2], T1[:, 0:2])
                    nc.vector.tensor_tensor(T2[:, 2:], T1[:, 2:c1], T1[:, 0:c1 - 2], ADD)
                    nc.vector.tensor_copy(WT[:, 0:4], T2[:, 0:4])
                    nc.vector.tensor_tensor(WT[:, 4:], T2[:, 4:], Z[:, 0:c1 - 4], ADD)
                else:
                    nc.vector.tensor_tensor(T1[:, c0:c1], Z[:, c0:c1], Z[:, c0 - 1:c1 - 1], ADD)
                    nc.vector.tensor_tensor(T2[:], T1[:, c0:c1], T1[:, c0 - 2:c1 - 2], ADD)
                    nc.vector.tensor_tensor(WT[:], T2[:], Z[:, c0 - 4:c1 - 4], ADD)
                nc.vector.tensor_tensor(V[0:WIN_ROWS, c0:c1], WT[:], coefr[:, c0:c1], MUL)

                ps2 = p2pool.tile([M2, NTC], f32)
                nc.tensor.matmul(ps2[:], ssb[:], V[:, c0:c1],
                                 start=True, stop=True)
                GRP = COLS // 4  # 1664, a whole number of dialogs
                for (glo, ghi) in [(c0, min(c1, (c0 // GRP + 1) * GRP)),
                                   ((c0 // GRP + 1) * GRP, c1)]:
                    if glo >= ghi:
                        continue
                    g = glo // GRP
                    nc.scalar.copy(tGP[32 * g:32 * g + NCLS, glo - g * GRP:ghi - g * GRP],
                                   ps2[0:NCLS, glo - c0:ghi - c0])
                    nc.scalar.copy(tOP[32 * g:32 * g + NCLS, glo - g * GRP:ghi - g * GRP],
                                   ps2[32:32 + NCLS, glo - c0:ghi - c0])

            # win2: 5-tap window of packed g0 (all 4 groups in one op per stage)
            GRP = COLS // 4
            NR = 96 + NCLS  # rows 0..102 cover the 4 groups
            gs1 = gpool.tile([NR, GRP], f32, tag="gs1")
            gs2 = gpool.tile([NR, GRP], f32, tag="gs2")
            gwt = gpool.tile([NR, GRP], f32, tag="gwt")
            gp = tGP[0:NR]
            nc.vector.tensor_copy(gs1[:, 0:1], gp[:, 0:1])
            nc.vector.tensor_tensor(gs1[:, 1:GRP], gp[:, 1:GRP], gp[:, 0:GRP - 1], ADD)
            nc.vector.tensor_copy(gs2[:, 0:2], gs1[:, 0:2])
            nc.vector.tensor_tensor(gs2[:, 2:GRP], gs1[:, 2:GRP], gs1[:, 0:GRP - 2], ADD)
            nc.vector.tensor_copy(gwt[:, 0:4], gs2[:, 0:4])
            nc.vector.tensor_tensor(gwt[:, 4:GRP], gs2[:, 4:GRP], gp[:, 0:GRP - 4], ADD)
            # packed out += packed W(g0) (single aligned add over all groups)
            nc.vector.tensor_tensor(tOP[0:NR, :], tOP[0:NR, :], gwt[:], ADD)
            for g in range(4):
                nc.sync.dma_start(y_d[:, g * GRP:(g + 1) * GRP],
                                  tOP[32 * g:32 * g + NCLS, :])
    nc.compile()
    return nc


def _get_compiled():
    global _COMPILED
    if _COMPILED is None:
        _COMPILED = _build()
    return _COMPILED


def _run(in_maps, trace=False):
    from concourse.bass_utils import run_bass_kernel_spmd
    nc = _get_compiled()
    return run_bass_kernel_spmd(nc, in_maps, list(range(NCORES)), trace=trace)


def kernel(**inputs) -> np.ndarray:
    in_maps, unshard_info = _host_prep(**inputs)
    res = _run(in_maps)
    out = np.zeros((N, NCLS), dtype=np.float32)
    for c in range(NCORES):
        nodes_real, cols_real = unshard_info[c]
        out[nodes_real] = res.results[c]["y"][:, cols_real].T
    return out



# revision 9
# speedup vs baseline: 1.8121x; 1.8121x over previous
"""Trainium2 Bass kernel for nn_DIAGCN (RGCN + GraphConv + classifier over
block-diagonal dialog graphs), SPMD over 8 NeuronCores.

Strategy
--------
The dialog graph is a causal 5-tap window (edges i -> i+o, o = 0..4, within
each 100-utterance dialog), and relation_type(i,j) = spk[i]*spk[j] with spk
derived from self-edges.  Every per-node linear map commutes with both the
window sum W(.) (row-mixing) and per-node diagonal scalings (row scaling), so
the whole network folds into 7-wide channels:

    y   = W(g0) + f0 + const
    g0  = rootA + ic0.*W(q0A) + (ic0-ic0s).*W(m0A) + ic1s.*W(m1A)
    f0  = same with B-weights
    m{0,1} = spk.*u{0,1},  q0 = (1-spk).*u0,  u{0,1}{A,B} = x@(w_rel{0,1}@w{A,B})
    wA = w_gc_rel@w_clf,  wB = w_gc_root@w_clf
    roots = x @ (w_root@w{A,B} [+ w_skip@w_clf])
    const = cA*W(mask) + cBc*mask  (all bias terms, host-precomputed)

Device work per core: one [1024 -> 56] fp16 matmul over x; ONE DVE op per
tile evacuates all 56 PSUM rows to fp16 SBUF with the spk masks applied
(root rows multiply by the data mask); 5-tap shift-tree windows on DVE (fp16
2x mode); the coef multiply on the otherwise-idle GpSimd engine; a [->39]
selection matmul pair (lagged one tile so the PE never stalls on the DVE
chain) reduces channels to packed g0/f0 rows; Act evacuates those; a second
packed window over g0 runs on two half planes (quarters as partition rows,
so its DVE cost is 1664 columns per op) - one mid-loop, one in the tail.
All HBM traffic is fp16, halving the memory-bound DMA bytes vs f32, with
8 KiB contiguous per partition per xt DMA.

Layout: nodes sharded by dialog (no cross-core edges), 64 padded dialogs per
core; each dialog stored as 4 zero "gap" columns + 100 data columns so window
sums never leak across dialogs.
"""
import numpy as np

# ---------------------------------------------------------------- constants
B, L, FUT = 500, 100, 4
N = B * L
IN, HID, NCLS = 1024, 512, 7
NCORES = 8
GAP = 4
DLG = L + GAP            # 104 columns per dialog
DPC = 64                 # padded dialogs per core
COLS = DPC * DLG         # 6656 columns per core
NT = 13                  # column tiles
NTC = COLS // NT         # 512
KB = IN // 128           # 8 contraction blocks
M = 56                   # Wbig columns (psum partitions used)
M2 = 39                  # S-matmul output columns
GRP = COLS // 4          # 1664 columns per packed win2 group (16 dialogs)
WROWS = 42               # windowed rows

D_COUNTS = [63, 63, 63, 63, 62, 62, 62, 62]
D_STARTS = np.concatenate([[0], np.cumsum(D_COUNTS)])[:-1]

# psum/Z row map (dense): 0:7 u0A, 7:14 u0B (-> m0 = spk.*u0),
# 14:21 u1A, 21:28 u1B (-> m1 = spk.*u1), 28:35 u0A dup, 35:42 u0B dup
# (-> q0 = (1-spk).*u0), 42:49 rootA, 49:56 rootB (mask-scaled copy).


def _data_cols():
    d = np.arange(DPC)[:, None]
    u = np.arange(L)[None, :]
    return d * DLG + GAP + u  # [DPC, L]


# ---------------------------------------------------------------- host prep
def _check_graph(edges, relation_type):
    i = np.arange(L)[:, None]
    off = np.arange(FUT + 1)[None, :]
    tl = i + off
    valid = tl < L
    sl = np.broadcast_to(i, tl.shape)[valid]
    tl = tl[valid]
    base = (np.arange(B) * L)[:, None]
    src = (base + sl[None, :]).reshape(-1)
    tgt = (base + tl[None, :]).reshape(-1)
    if edges.shape != (2, src.size) or not (
        np.array_equal(edges[0], src) and np.array_equal(edges[1], tgt)
    ):
        raise ValueError("edge structure does not match the DIAGCN pattern")
    sel = edges[0] == edges[1]
    spk = np.zeros(N, dtype=np.float64)
    spk[edges[0][sel]] = relation_type[sel]
    return spk


def _win_cols(v):
    """5-tap causal window along the padded column axis."""
    out = np.zeros_like(v)
    for o in range(FUT + 1):
        if o:
            out[o:] += v[:v.shape[0] - o]
        else:
            out += v
    return out


def _host_prep(x, edges, relation_type, w_rel, w_root, b_rgcn,
               w_gc_rel, w_gc_root, b_gc, w_skip, b_skip, w_clf, b_clf):
    x = np.asarray(x, dtype=np.float32)
    edges = np.asarray(edges)
    relation_type = np.asarray(relation_type)
    spk = _check_graph(edges, relation_type)

    tgt = edges[1]
    c1 = np.bincount(tgt[relation_type == 1], minlength=N).astype(np.float64)
    c0 = np.bincount(tgt[relation_type == 0], minlength=N).astype(np.float64)
    ic0 = 1.0 / np.maximum(c0, 1.0)
    ic1 = 1.0 / np.maximum(c1, 1.0)
    ic0s = ic0 * spk
    ic1s = ic1 * spk

    f8 = lambda a: np.asarray(a, dtype=np.float64)
    w_rel, w_root, w_gc_rel, w_gc_root, w_skip, w_clf = map(
        f8, (w_rel, w_root, w_gc_rel, w_gc_root, w_skip, w_clf))
    b_rgcn, b_gc, b_skip, b_clf = map(f8, (b_rgcn, b_gc, b_skip, b_clf))

    wA = w_gc_rel @ w_clf
    wB = w_gc_root @ w_clf
    w0A, w1A = w_rel[0] @ wA, w_rel[1] @ wA
    w0B, w1B = w_rel[0] @ wB, w_rel[1] @ wB
    Wbig = np.zeros((IN, M), dtype=np.float64)
    Wbig[:, 0:7] = w0A
    Wbig[:, 7:14] = w0B
    Wbig[:, 14:21] = w1A
    Wbig[:, 21:28] = w1B
    Wbig[:, 28:35] = w0A
    Wbig[:, 35:42] = w0B
    Wbig[:, 42:49] = w_root @ wA
    Wbig[:, 49:56] = w_root @ wB + w_skip @ w_clf
    # [128 partitions, KB, M]: partition p holds weight rows {k*128+p}
    Wbig = np.ascontiguousarray(
        Wbig.astype(np.float16).reshape(KB, 128, M).swapaxes(0, 1))

    # S matrices: channel reduction onto ps2 rows {0:7}=g0, {32:39}=f0
    # slot 0 reduces the windowed+coef'd V rows, slot 1 picks the roots.
    S = np.zeros((128, 2, M2), dtype=np.float16)
    for i in range(7):
        for r in (0, 14, 28):               # m0, m1, q0 (windowed, coef'd)
            S[r + i, 0, i] = 1.0            # A-channels -> g0
            S[r + 7 + i, 0, 32 + i] = 1.0   # B-channels -> f0
        S[42 + i, 1, i] = 1.0               # rootA -> g0 (from Z plane)
        S[49 + i, 1, 32 + i] = 1.0          # rootB -> f0

    cA = b_rgcn @ wA                        # [7]
    cBc = b_rgcn @ wB + (b_gc + b_skip) @ w_clf + b_clf

    dc = _data_cols()
    mask_col = np.zeros(COLS, dtype=np.float64)
    mask_col[dc.reshape(-1)] = 1.0
    wmask = _win_cols(mask_col)
    # out_const = cA (x) W(mask) + cBc (x) mask; two packed half planes,
    # quarters 2h, 2h+1 at rows 0:7, 32:39.
    constp = np.zeros((2, M2, GRP), dtype=np.float16)
    for g in range(4):
        seg = slice(g * GRP, (g + 1) * GRP)
        constp[g // 2, 32 * (g % 2):32 * (g % 2) + 7] = (
            cA[:, None] * wmask[None, seg] + cBc[:, None] * mask_col[None, seg]
        ).astype(np.float16)

    in_maps = []
    unshard_info = []
    for c in range(NCORES):
        nd = D_COUNTS[c]
        g0 = D_STARTS[c]
        cols_real = dc[:nd].reshape(-1)
        nodes_real = g0 * L + np.arange(nd * L)

        xt = np.zeros((IN, COLS), dtype=np.float16)
        xt[:, cols_real] = x[nodes_real].T.astype(np.float16)
        # dram layout [NT, 128, KB*NTC]: one DMA per tile, 8 KiB contiguous
        # per partition (8 k-blocks x 512 cols fp16)
        xts = np.ascontiguousarray(
            xt.reshape(KB, 128, NT, NTC).transpose(2, 1, 0, 3)
            .reshape(NT, 128, KB * NTC))

        def vec_to_cols(v):
            out = np.zeros(COLS, dtype=np.float64)
            out[cols_real] = v[nodes_real]
            return out

        spk_c = vec_to_cols(spk)
        ic0_c = vec_to_cols(ic0)
        ic0s_c = vec_to_cols(ic0s)
        ic1s_c = vec_to_cols(ic1s)

        spkrep = np.zeros((M, COLS), dtype=np.float16)
        spkrep[0:14] = spk_c.astype(np.float16)
        spkrep[14:28] = spk_c.astype(np.float16)
        spkrep[28:42] = ((1.0 - spk_c) * mask_col).astype(np.float16)
        spkrep[42:56] = mask_col.astype(np.float16)
        coefrep = np.zeros((WROWS, COLS), dtype=np.float16)
        coefrep[0:14] = (ic0_c - ic0s_c).astype(np.float16)
        coefrep[14:28] = ic1s_c.astype(np.float16)
        coefrep[28:42] = ic0_c.astype(np.float16)

        in_maps.append(dict(
            xt=xts, wbig=Wbig, smat=S,
            spkrep=spkrep, coefrep=coefrep, constp=constp,
        ))
        unshard_info.append((nodes_real, cols_real))
    return in_maps, unshard_info


# ---------------------------------------------------------------- bass kernel
_COMPILED = None


def _build():
    import concourse.bass as bass
    from concourse import bacc
    import concourse.mybir as mybir
    from concourse.tile import TileContext

    f16 = mybir.dt.float16
    f32 = mybir.dt.float32
    ADD = mybir.AluOpType.add
    MUL = mybir.AluOpType.mult

    nc = bacc.Bacc("TRN2", target_bir_lowering=False, debug=False,
                   num_devices=NCORES)
    xt_d = nc.dram_tensor("xt", [NT, 128, KB * NTC], f16, kind="ExternalInput")
    wbig_d = nc.dram_tensor("wbig", [128, KB, M], f16, kind="ExternalInput")
    smat_d = nc.dram_tensor("smat", [128, 2, M2], f16, kind="ExternalInput")
    spkrep_d = nc.dram_tensor("spkrep", [M, COLS], f16, kind="ExternalInput")
    coefrep_d = nc.dram_tensor("coefrep", [WROWS, COLS], f16, kind="ExternalInput")
    constp_d = nc.dram_tensor("constp", [2, M2, GRP], f16, kind="ExternalInput")
    y_d = nc.dram_tensor("y", [NCLS, COLS], f16, kind="ExternalOutput")

    with TileContext(nc) as tc:
        with (
            tc.tile_pool(name="const", bufs=1) as cpool,
            tc.tile_pool(name="xin", bufs=4) as xpool,
            tc.tile_pool(name="wrk", bufs=2) as wpool,
            tc.tile_pool(name="g2", bufs=1) as gpool,
            tc.tile_pool(name="psum", bufs=4, space="PSUM") as ppool,
            tc.tile_pool(name="psum2", bufs=3, space="PSUM") as p2pool,
        ):
            wsb = cpool.tile([128, KB, M], f16)
            nc.scalar.dma_start(wsb[:], wbig_d[:])
            ssb = cpool.tile([128, 2, M2], f16)
            nc.scalar.dma_start(ssb[:], smat_d[:])

            tZ = cpool.tile([M, COLS], f16)
            tT1 = cpool.tile([WROWS, COLS], f16)
            tSPK = cpool.tile([M, COLS], f16)
            tCF = cpool.tile([WROWS, COLS], f16)
            # packed half planes: quarters 2h, 2h+1 at rows 0:7, 32:39
            tGP = [cpool.tile([M2, GRP], f16, name=f"tGP{h}") for h in range(2)]
            tFP = [cpool.tile([M2, GRP], f16, name=f"tFP{h}") for h in range(2)]
            tCP = [cpool.tile([M2, GRP], f16, name=f"tCP{h}") for h in range(2)]
            tYP = [cpool.tile([M2, GRP], f16, name=f"tYP{h}") for h in range(2)]
            nc.scalar.dma_start(tSPK[:], spkrep_d[:])
            nc.scalar.dma_start(tCF[:], coefrep_d[:])
            nc.scalar.dma_start(tCP[0][:], constp_d[0])
            nc.scalar.dma_start(tCP[1][:], constp_d[1])

            T1 = tT1[0:WROWS]

            def win2(h):
                """Second 5-tap window over packed g0 half-plane h, then
                y = W(g0) + f0 + const, then DMA out both quarters."""
                w1 = gpool.tile([M2, GRP], f16, tag=f"w1{h}", name=f"w1_{h}")
                w2 = gpool.tile([M2, GRP], f16, tag=f"w2{h}", name=f"w2_{h}")
                gp, fp, cp, yp = tGP[h][:], tFP[h][:], tCP[h][:], tYP[h][:]
                nc.vector.tensor_copy(w1[:, 0:1], gp[:, 0:1])
                nc.vector.tensor_tensor(w1[:, 1:GRP], gp[:, 1:GRP], gp[:, 0:GRP - 1], ADD)
                nc.vector.tensor_copy(w2[:, 0:2], w1[:, 0:2])
                nc.vector.tensor_tensor(w2[:, 2:GRP], w1[:, 2:GRP], w1[:, 0:GRP - 2], ADD)
                nc.vector.tensor_copy(yp[:, 0:4], w2[:, 0:4])
                nc.vector.tensor_tensor(yp[:, 4:GRP], w2[:, 4:GRP], gp[:, 0:GRP - 4], ADD)
                nc.vector.tensor_tensor(yp[:], yp[:], fp, ADD)
                nc.vector.tensor_tensor(yp[:], yp[:], cp, ADD)
                for q in range(2):
                    g = 2 * h + q
                    nc.sync.dma_start(y_d[:, g * GRP:(g + 1) * GRP],
                                      tYP[h][32 * q:32 * q + NCLS, :])

            prev = None  # (V, c0, c1) of previous tile, awaiting S-matmuls
            for t in range(NT):
                c0, c1 = t * NTC, (t + 1) * NTC
                xt_t = xpool.tile([128, KB, NTC], f16)
                nc.sync.dma_start(xt_t[:], xt_d[t])
                ps = ppool.tile([M, NTC], f32)
                for k in range(KB):
                    nc.tensor.matmul(ps[:], wsb[:, k, :], xt_t[:, k, :],
                                     start=(k == 0), stop=(k == KB - 1))
                if prev is not None:
                    _sreduce(nc, p2pool, prev, ssb, tZ, tGP, tFP)

                # PSUM -> SBUF fp16 with per-row masks applied (one DVE op)
                nc.vector.tensor_tensor(tZ[:, c0:c1], ps[:], tSPK[:, c0:c1], MUL)

                # 5-tap causal window as a shift tree (rows 0:42):
                #   t1 = z + sh1(z); t2 = t1 + sh2(t1); wt = t2 + sh4(z)
                T2 = wpool.tile([WROWS, NTC], f16, tag="T2")
                WT = wpool.tile([WROWS, NTC], f16, tag="WT")
                V = wpool.tile([WROWS, NTC], f16, tag="V")
                Zw = tZ[0:WROWS]
                if t == 0:
                    nc.vector.tensor_copy(T1[:, 0:1], Zw[:, 0:1])
                    nc.vector.tensor_tensor(T1[:, 1:c1], Zw[:, 1:c1], Zw[:, 0:c1 - 1], ADD)
                    nc.vector.tensor_copy(T2[:, 0:2], T1[:, 0:2])
                    nc.vector.tensor_tensor(T2[:, 2:], T1[:, 2:c1], T1[:, 0:c1 - 2], ADD)
                    nc.vector.tensor_copy(WT[:, 0:4], T2[:, 0:4])
                    nc.vector.tensor_tensor(WT[:, 4:], T2[:, 4:], Zw[:, 0:c1 - 4], ADD)
                else:
                    nc.vector.tensor_tensor(T1[:, c0:c1], Zw[:, c0:c1], Zw[:, c0 - 1:c1 - 1], ADD)
                    nc.vector.tensor_tensor(T2[:], T1[:, c0:c1], T1[:, c0 - 2:c1 - 2], ADD)
                    nc.vector.tensor_tensor(WT[:], T2[:], Zw[:, c0 - 4:c1 - 4], ADD)
                # coef multiply on the otherwise-idle GpSimd engine
                nc.gpsimd.tensor_tensor(V[:], WT[:], tCF[:, c0:c1], MUL)

                prev = (V, c0, c1)
                if t == 7:
                    win2(0)   # quarters 0,1 complete after _sreduce of tile 6
            _sreduce(nc, p2pool, prev, ssb, tZ, tGP, tFP)
            win2(1)
    nc.compile()
    return nc


def _sreduce(nc, p2pool, prev, ssb, tZ, tGP, tFP):
    """Lagged channel reduction for the previous tile: V + Z-roots -> ps2,
    then Act evacuates ps2 into the packed g0/f0 half planes."""
    import concourse.mybir as mybir
    f32 = mybir.dt.float32
    V, c0, c1 = prev
    ps2 = p2pool.tile([M2, NTC], f32, name="ps2")
    nc.tensor.matmul(ps2[:], ssb[0:WROWS, 0, :], V[:], start=True, stop=False)
    nc.tensor.matmul(ps2[:], ssb[0:M, 1, :], tZ[:, c0:c1], start=False, stop=True)
    lo_g, hi_g = c0 // GRP, (c1 - 1) // GRP
    for g in range(lo_g, hi_g + 1):
        glo, ghi = max(c0, g * GRP), min(c1, (g + 1) * GRP)
        h, q = g // 2, g % 2
        dst = slice(glo - g * GRP, ghi - g * GRP)
        src = slice(glo - c0, ghi - c0)
        nc.scalar.copy(tGP[h][32 * q:32 * q + NCLS, dst], ps2[0:NCLS, src])
        nc.scalar.copy(tFP[h][32 * q:32 * q + NCLS, dst], ps2[32:32 + NCLS, src])


def _get_compiled():
    global _COMPILED
    if _COMPILED is None:
        _COMPILED = _build()
    return _COMPILED


def _run(in_maps, trace=False):
    from concourse.bass_utils import run_bass_kernel_spmd
    nc = _get_compiled()
    return run_bass_kernel_spmd(nc, in_maps, list(range(NCORES)), trace=trace)


def kernel(**inputs) -> np.ndarray:
    in_maps, unshard_info = _host_prep(**inputs)
    res = _run(in_maps)
    out = np.zeros((N, NCLS), dtype=np.float32)
    for c in range(NCORES):
        nodes_real, cols_real = unshard_info[c]
        out[nodes_real] = res.results[c]["y"][:, cols_real].T.astype(np.float32)
    return out
